# revision 12
# baseline (speedup 1.0000x reference)
"""LoRA layer kernel for Trainium2 (8 NeuronCores, data-parallel).

Computes out = SCALING * (x @ A^T) @ B^T for x [4, 8192, 1024],
lora_A [4, 1024], lora_B [1024, 4], SCALING = 0.25.

Strategy (per core, shard = 4096 rows x 1024 features), fp16 wire format:
  - x rows sharded across 8 cores; A/B replicated on every core.
  - The host pre-casts x to fp16 and pre-transposes each shard to a
    feature-major grouped layout [P=128, G=8, C=8, M=512] so every DMA
    load lands 8 KiB contiguous per partition and the kernel needs no
    on-chip transposes at all.
  - The rank-4 matrices are replicated 32x on the host so both matmul
    stages run [K=128, M=128, N=512] on a fully lit 128x128 PE array:
    a rank-4 lhsT (M=4) looks idle to the PE activity monitor, which
    re-throttles the PE clock to 1.2 GHz; padded to 128 it stays at
    2.4 GHz and fast-weight-load engages.  at_rep carries a 1/32 scale
    so the 32 redundant replicas sum back to the true product.
  - Per 512-row group: one 1 MiB load (SP HWDGE ring), 8 accumulating
    matmuls ht_rep = at_rep^T x^T, DVE evac to fp16, 8 output matmuls
    out = ht_rep^T bt_rep, PSUM evacuation alternating DVE/ScalarE,
    one 1 MiB fp16 store per group on the ACT HWDGE ring.  Constants
    also ride the ACT ring so nothing blocks the first group.
  - Host converts the fp16 result back to f32 and un-permutes rows.
"""

import sys

for _p in (
    "/root/.axon_site",
    "/root/.axon_site/_ro/trn_rl_repo",
    "/root/.axon_site/_ro/pypackages",
):
    if _p not in sys.path:
        sys.path.insert(0, _p)

from contextlib import ExitStack

import numpy as np

N_CORES = 8
D_IN = 1024
D_OUT = 1024
RANK = 4
REP = 32           # replicas of the rank-4 factors to fill 128 partitions
ROWS_TOTAL = 4 * 8192
ROWS_PER_CORE = ROWS_TOTAL // N_CORES  # 4096
SCALING = 1.0 / RANK

P = 128            # partitions
C = D_IN // P      # 8 feature chunks
GROUP_ROWS = 512   # rows per pipeline iteration
N_GROUPS = ROWS_PER_CORE // GROUP_ROWS  # 8
J = GROUP_ROWS // P  # 4 row subtiles per group
OCH = 512          # output columns per PSUM bank


def emit_lora(tc, x_ap, at_ap, bt_ap, out_ap):
    """Emit the LoRA kernel IR for one core's shard.

    x_ap  : DRAM [P, N_GROUPS, C, GROUP_ROWS] fp16,
            x_ap[p, g, c, m] = x[g*512 + m, c*128 + p]
    at_ap : DRAM [P, C, P] fp16, at[p, c, 32k+r] = A[r, c*128 + p] / 32
    bt_ap : DRAM [P, D_OUT] fp16, bt[32k+r, o] = SCALING * B[o, r]
    out_ap: DRAM [P, N_GROUPS, J, D_OUT] fp16, row = g*512 + j*128 + p
    """
    import concourse.mybir as mybir

    nc = tc.nc
    f32 = mybir.dt.float32
    f16 = mybir.dt.float16
    ctx = tc._ctx  # ExitStack owned by caller

    consts = ctx.enter_context(tc.tile_pool(name="consts", bufs=1))
    xtpool = ctx.enter_context(tc.tile_pool(name="xt", bufs=1))
    htpool = ctx.enter_context(tc.tile_pool(name="ht", bufs=2))
    opool = ctx.enter_context(tc.tile_pool(name="osb", bufs=4))
    ps_ht = ctx.enter_context(tc.tile_pool(name="ps_ht", bufs=2, space="PSUM"))
    ps_o = ctx.enter_context(tc.tile_pool(name="ps_o", bufs=3, space="PSUM"))

    # Queue order on the SP HWDGE ring: group 0's x first (longest pole for
    # the first real matmul), then the small at/bt, then the remaining
    # loads, so the SDMA engines never idle on the load side.
    xts = [
        xtpool.tile([P, C, GROUP_ROWS], f16, name=f"xt{g}")
        for g in range(N_GROUPS)
    ]
    at_sb = consts.tile([P, C, P], f16)
    nc.sync.dma_start(at_sb[:], at_ap[:])
    # Group 0's load is split in half so its first rank matmuls can start
    # one DMA-completion latency earlier.
    nc.sync.dma_start(xts[0][:, 0 : C // 2], x_ap[:, 0, 0 : C // 2])
    nc.sync.dma_start(xts[0][:, C // 2 : C], x_ap[:, 0, C // 2 : C])
    bt_sb = consts.tile([P, D_OUT], f16)
    nc.sync.dma_start(bt_sb[:], bt_ap[:])
    for g in range(1, N_GROUPS):
        nc.sync.dma_start(xts[g][:], x_ap[:, g])

    # Warmup matmuls gated only on the small at load: they run while the
    # first x load is still in flight, so the PE activity monitor has the
    # clock at 2.4 GHz before the first real matmul (result never read).
    warm_ps = ps_o.tile([P, D_OUT], f32, name="o_ps")
    for w in range(8):
        nc.tensor.matmul(
            warm_ps[:, 0:OCH],
            lhsT=at_sb[:, 0, :],
            rhs=at_sb[:, 0:4, :],
            start=True,
            stop=True,
        )

    def rank_stage(g):
        # ht_rep[32k+r, m] += sum_f at_rep[c*128+f, 32k+r] * x^T[c*128+f, m]
        xt = xts[g]
        ht_ps = ps_ht.tile([P, GROUP_ROWS], f32, name="ht_ps")
        for c in range(C):
            nc.tensor.matmul(
                ht_ps[:],
                lhsT=at_sb[:, c, :],
                rhs=xt[:, c, :],
                start=(c == 0),
                stop=(c == C - 1),
            )
        ht_sb = htpool.tile([P, GROUP_ROWS], f16, name="ht_sb")
        if g % 2 == 0:
            nc.scalar.copy(ht_sb[:], ht_ps[:])
        else:
            nc.vector.tensor_copy(ht_sb[:], ht_ps[:])
        return ht_sb

    def out_stage(g, ht_sb):
        o_sb = opool.tile([P, J, D_OUT], f16, name="o_sb")
        for j in range(J):
            # Two bank-aligned matmuls fill one 2-bank PSUM tile; a single
            # evacuation per j amortizes the ~700ns fixed cost of a
            # PSUM-read copy (evacs alternate between ScalarE and DVE).
            o_ps = ps_o.tile([P, D_OUT], f32, name="o_ps")
            for o2 in range(D_OUT // OCH):
                # out[m, o] = sum_{32k+r} ht_rep[32k+r, m] * bt_rep[32k+r, o]
                nc.tensor.matmul(
                    o_ps[:, o2 * OCH : (o2 + 1) * OCH],
                    lhsT=ht_sb[:, j * P : (j + 1) * P],
                    rhs=bt_sb[:, o2 * OCH : (o2 + 1) * OCH],
                    start=True,
                    stop=True,
                )
            dst = o_sb[:, j, :]
            if j % 2 == 0:
                nc.vector.tensor_copy(dst, o_ps[:])
            else:
                nc.scalar.copy(dst, o_ps[:])

        # Stores ride the SWDGE (gpsimd) ring: ScalarE stays free for PSUM
        # evacuation and a store waiting on its evacs never head-of-line
        # blocks the HWDGE load ring.  The final group stores in two halves
        # so the last transfer is small and the tail drains sooner.
        if g == N_GROUPS - 1:
            nc.gpsimd.dma_start(out_ap[:, g, 0 : J // 2], o_sb[:, 0 : J // 2])
            nc.gpsimd.dma_start(out_ap[:, g, J // 2 : J], o_sb[:, J // 2 : J])
        else:
            nc.gpsimd.dma_start(out_ap[:, g], o_sb[:])

    # Software-pipelined: group g's rank stage is issued before group g-1's
    # output stage, so the ht evacuation of group g-1 (DVE) overlaps group
    # g's rank matmuls instead of stalling the strict-FIFO PE queue.
    pending = None
    for g in range(N_GROUPS):
        ht_sb = rank_stage(g)
        if pending is not None:
            out_stage(g - 1, pending)
        pending = ht_sb
    out_stage(N_GROUPS - 1, pending)


def build_nc():
    import concourse.mybir as mybir
    import concourse.tile as tile
    from concourse import bacc

    f16 = mybir.dt.float16
    nc = bacc.Bacc("TRN2", target_bir_lowering=False, debug=False)
    x_d = nc.dram_tensor(
        "x", [P, N_GROUPS, C, GROUP_ROWS], f16, kind="ExternalInput"
    ).ap()
    at_d = nc.dram_tensor("at", [P, C, P], f16, kind="ExternalInput").ap()
    bt_d = nc.dram_tensor("bt", [P, D_OUT], f16, kind="ExternalInput").ap()
    out_d = nc.dram_tensor(
        "out", [P, N_GROUPS, J, D_OUT], f16, kind="ExternalOutput"
    ).ap()

    with tile.TileContext(nc) as tc:
        with ExitStack() as ctx:
            tc._ctx = ctx
            emit_lora(tc, x_d, at_d, bt_d, out_d)
    nc.compile()
    return nc


def host_prep(lora_A, lora_B):
    # at[p, c, 32k+r] = A[r, c*P + p] / REP  (REP identical replicas, scaled
    # so the redundant 32-fold sum in the output matmul is exact)
    a = np.asarray(lora_A, dtype=np.float32) / REP  # [RANK, D_IN]
    atc = a.T.reshape(C, P, RANK).transpose(1, 0, 2)  # [P, C, RANK]
    at = np.tile(atc, (1, 1, REP)).astype(np.float16)  # [P, C, RANK*REP]
    # bt[32k+r, o] = SCALING * B[o, r]
    b = (np.asarray(lora_B, dtype=np.float32).T * SCALING).astype(np.float16)
    bt = np.tile(b, (REP, 1))  # [P, D_OUT]
    return np.ascontiguousarray(at), np.ascontiguousarray(bt)


def shard_x(x):
    """x [4, 8192, 1024] f32 -> per-core [P, G, C, M] fp16 feature-major."""
    x2 = np.asarray(x).astype(np.float16).reshape(N_CORES, ROWS_PER_CORE, D_IN)
    shards = []
    for i in range(N_CORES):
        xt = x2[i].T  # [D_IN, rows] ; xt[c*128+p, g*512+m]
        xdev = xt.reshape(C, P, N_GROUPS, GROUP_ROWS).transpose(1, 2, 0, 3)
        shards.append(np.ascontiguousarray(xdev))
    return shards


def unshard_out(results):
    """Per-core out [P, G, J, D_OUT] fp16 -> full [4, 8192, 1024] f32."""
    outs = []
    for r in results:
        o = r["out"]  # [P, N_GROUPS, J, D_OUT] fp16 ; row = g*512 + j*128 + p
        outs.append(o.transpose(1, 2, 0, 3).reshape(ROWS_PER_CORE, D_OUT))
    return np.concatenate(outs, axis=0).astype(np.float32).reshape(4, 8192, D_OUT)


_NC_CACHE = {}


def kernel(x, lora_A, lora_B):
    from concourse.bass_utils import run_bass_kernel_spmd

    if "nc" not in _NC_CACHE:
        _NC_CACHE["nc"] = build_nc()
    nc = _NC_CACHE["nc"]

    shards = shard_x(x)
    at, bt = host_prep(lora_A, lora_B)
    in_maps = [{"x": shards[i], "at": at, "bt": bt} for i in range(N_CORES)]
    res = run_bass_kernel_spmd(nc, in_maps, core_ids=list(range(N_CORES)))
    return unshard_out([res.results[i] for i in range(N_CORES)])
